# revision 10
# baseline (speedup 1.0000x reference)
"""Trainium2 Bass kernel for nn_CustomLoss_50843822850472.

Computes, for L2-normalized rows f of `features` [8192, 128]:
    sim = f @ f.T                       (diagonal excluded)
    E   = exp((sim - c)/TAU)            (c = shift center, host rescales)
    S_i = sum_j E_ij                    (total mass)
    loss = mean_i [ log(den_i) - log(num_i) ]

Rows are split across 8 NeuronCores (1024 rows/core, 8 blocks of 128).
Each core gets the full feature matrix pre-transposed to [D=128, N=8192]
fp16 and column-rotated by its row offset (SPMD: the diagonal block of
row-block m lands at local columns [m*128, m*128+128), inside chunk 0,
where an accumulate-matmul adds -60000*I so exp -> 0 and the diagonal
drops out of every reduction).

Per row-block m (PSUM chunks of 2048 cols, double-buffered):
  - chunks 0,1,3 and the first half of chunk 2 -> ACT:
    E = exp((sim - c)/TAU) -> fp16 (no accum: reductions are done by a
    DVE pairwise tree, so ACT runs at pure streaming rate)
  - second half of chunk 2 (1024 cols) -> DVE Schraudolph exp:
    bits = u16(sim*slope + B0); the f32->u16 convert saturates (negatives
    clamp to 0 = +0.0 in fp16) and rounds to nearest, so the fp16 view of
    the bits is exp((sim-c)/TAU) within ~3%.  This offloads ACT.
  - a 4-level pairwise fp16 add-tree on DVE (tensor_tensor, 2x mode)
    folds the row [128, 8192] -> [128, 512]; column g of the result is
    sum over {E[g + 512k]}.  The [128, 512] block is DMA'd out per block.

Host: S_row = sum of the 512 partials (fp64) * exp(c/TAU).  Detection:
any partial >= 0.8*exp((alpha-c)/TAU) means the row may contain a
positive pair (a positive contributes >= ~1x threshold to its group,
group background is ~0.03 for alpha=0.5); flagged rows (~180 of 8192)
are recomputed exactly in fp64 on the host, all other rows have P = 0
exactly (num = EPS), matching the reference.
"""
import sys

sys.path.insert(0, "/opt/trn_rl_repo")

import numpy as np

TAU = 0.07
EPS = 1e-10
DIAG_NEG = -60000.0

N = 8192
D = 128
NCORES = 8
R = N // NCORES          # rows per core
NBLK = R // 128          # row blocks per core
CHUNK = 2048             # columns per PSUM chunk (4 banks)
NCHUNK = N // CHUNK
DVE_COLS = 2048          # columns of chunk 2 converted on DVE
TREE_OUT = 2048          # tree output width per block
LOG2E = float(np.log2(np.e))
_CACHE = {}
LAST_RESULT = None
PROFILE = False


def _shift_center(alpha: float) -> float:
    # E = exp((sim - c)/TAU) must fit fp16 (and the Schraudolph bits must
    # stay below fp16-inf = 31744): sim <= ~1.0002 needs c >= ~0.23.
    return float(min(max(alpha, 0.30), 1.0))


def _build(alpha: float):
    import concourse.mybir as mybir
    from concourse import bacc, tile

    f32 = mybir.dt.float32
    f16 = mybir.dt.float16
    u16 = mybir.dt.uint16
    Alu = mybir.AluOpType

    c = _shift_center(alpha)
    bias = float(-c / TAU)
    slope = float(1024.0 * LOG2E / TAU)
    b0 = float(15360.0 - c * slope - 44.5)

    nc = bacc.Bacc(
        "TRN2", target_bir_lowering=False, debug=False, num_devices=NCORES
    )
    ft_d = nc.dram_tensor("ft", [128, N], f16, kind="ExternalInput")
    ident_d = nc.dram_tensor("ident", [128, 128], f16, kind="ExternalInput")
    negd_d = nc.dram_tensor("negd", [128, 128], f16, kind="ExternalInput")
    out_d = nc.dram_tensor(
        "treeS", [128, NBLK * TREE_OUT], f16, kind="ExternalOutput"
    )

    with tile.TileContext(nc) as tc:
        with (
            tc.tile_pool(name="sb", bufs=1) as sb,
            tc.tile_pool(name="ep", bufs=2) as ep,
            tc.tile_pool(name="tp", bufs=2) as tp,
            tc.tile_pool(name="ppa", bufs=2, space="PSUM") as ppa,
            tc.tile_pool(name="ppd", bufs=1, space="PSUM") as ppd,
        ):
            # DMA pieces aligned to the consumer chunk boundaries
            ft = sb.tile([128, N], f16)
            pieces = [(0, 1536), (1536, 3072), (3072, 4608), (4608, 6144),
                      (6144, 8192)]
            nc.sync.dma_start(ft[:, 0:1536], ft_d[:, 0:1536])
            ident = sb.tile([128, 128], f16)
            nc.scalar.dma_start(ident[:], ident_d[:])
            negd = sb.tile([128, 128], f16)
            nc.scalar.dma_start(negd[:], negd_d[:])
            for lo, hi in pieces[1:]:
                nc.sync.dma_start(ft[:, lo:hi], ft_d[:, lo:hi])

            biast = sb.tile([128, 1], f32)
            nc.vector.memset(biast[:], bias)

            # software-pipelined tree state from the previous block
            prev = None     # (E, t1a, m)

            def finish_tree(E, t1a, m, split=False):
                # T1b: right half pair (cvt cols + chunk3 cols)
                t1b = tp.tile([128, 2048], f16, tag="t1b")
                nc.vector.tensor_tensor(
                    out=t1b[:], in0=E[:, 4096:6144], in1=E[:, 6144:8192],
                    op=Alu.add)
                if not split:
                    t2 = tp.tile([128, TREE_OUT], f16, tag="t2")
                    nc.vector.tensor_tensor(
                        out=t2[:], in0=t1a[:], in1=t1b[:], op=Alu.add)
                    nc.sync.dma_start(
                        out_d[:, m * TREE_OUT:(m + 1) * TREE_OUT], t2[:])
                    return
                # last block: halve T2 so the first DMA overlaps the second
                t2 = tp.tile([128, TREE_OUT], f16, tag="t2")
                for h in range(2):
                    lo, hi = h * 1024, (h + 1) * 1024
                    nc.vector.tensor_tensor(
                        out=t2[:, lo:hi], in0=t1a[:, lo:hi],
                        in1=t1b[:, lo:hi], op=Alu.add)
                    nc.sync.dma_start(
                        out_d[:, m * TREE_OUT + lo:m * TREE_OUT + hi],
                        t2[:, lo:hi])

            # per-block column layout: 4 ACT chunks of 1536 ([0:6144]) and
            # 2 DVE cvt chunks of 1024 ([6144:8192]).  PSUM: 4*3 + 2 = 8
            # banks, so the block m+1 chunk-0 buffer is freed by an EARLY
            # ACT chunk of block m, not by the last consumer (no boundary
            # stall).
            ACHUNK = 1536
            DCHUNK = 1024

            def mains(ps, m, lo, cols):
                d0 = m * 128
                for q in range(cols // 512):
                    nc.tensor.matmul(
                        ps[:, q * 512:(q + 1) * 512],
                        lhsT=ft[:, m * 128:(m + 1) * 128],
                        rhs=ft[:, lo + q * 512:lo + (q + 1) * 512],
                        start=True,
                        stop=not (lo + q * 512 <= d0 < lo + (q + 1) * 512),
                    )
                    if lo + q * 512 <= d0 < lo + (q + 1) * 512:
                        # accumulate -60000 onto the diagonal 128 cols
                        nc.tensor.matmul(
                            ps[:, d0 - lo:d0 - lo + 128],
                            lhsT=ident[:], rhs=negd[:],
                            start=False, stop=True,
                        )

            def act_exp(E, ps, lo, cols):
                nc.scalar.activation(
                    E[:, lo:lo + cols], ps[:],
                    mybir.ActivationFunctionType.Exp,
                    scale=float(1.0 / TAU), bias=biast[:],
                )

            def dve_exp(E, ps, lo, cols):
                # Schraudolph exp: u16(sim*slope + b0), saturating RNE
                nc.vector.tensor_scalar(
                    out=E[:, lo:lo + cols].bitcast(u16),
                    in0=ps[:],
                    scalar1=slope, scalar2=b0,
                    op0=Alu.mult, op1=Alu.add,
                )

            for m in range(NBLK - 1):
                E = ep.tile([128, N], f16)
                # ACT chunks 0,1 + first DVE chunk fill
                psa = []
                for k in range(2):
                    ps = ppa.tile([128, ACHUNK], f32, tag="psa")
                    mains(ps, m, k * ACHUNK, ACHUNK)
                    psa.append(ps)
                pd1 = ppd.tile([128, DCHUNK], f32, tag="psd")
                mains(pd1, m, 6144, DCHUNK)
                for k in range(2):
                    act_exp(E, psa[k], k * ACHUNK, ACHUNK)
                # ACT chunks 2,3 + second DVE chunk fill
                psb = []
                for k in range(2, 4):
                    ps = ppa.tile([128, ACHUNK], f32, tag="psa")
                    mains(ps, m, k * ACHUNK, ACHUNK)
                    psb.append(ps)
                pd2 = ppd.tile([128, DCHUNK], f32, tag="psd")
                mains(pd2, m, 6144 + DCHUNK, DCHUNK)
                if prev is not None:
                    finish_tree(*prev)
                dve_exp(E, pd1, 6144, DCHUNK)
                for k in range(2, 4):
                    act_exp(E, psb[k - 2], k * ACHUNK, ACHUNK)
                dve_exp(E, pd2, 6144 + DCHUNK, DCHUNK)
                # T1a: left half pair, ready once ACT chunks 0-2 are written
                t1a = tp.tile([128, 2048], f16, tag="t1a")
                nc.vector.tensor_tensor(
                    out=t1a[:], in0=E[:, 0:2048], in1=E[:, 2048:4096],
                    op=Alu.add)
                prev = (E, t1a, m)

            # last block: right side (chunks 2,3 + cvt cols) first, so T1b
            # overlaps the remaining ACT work and only T1a+T2 trail the
            # final EXP
            m = NBLK - 1
            E = ep.tile([128, N], f16)
            psb = []
            for k in range(2, 4):
                ps = ppa.tile([128, ACHUNK], f32, tag="psa")
                mains(ps, m, k * ACHUNK, ACHUNK)
                psb.append(ps)
            pd1 = ppd.tile([128, DCHUNK], f32, tag="psd")
            mains(pd1, m, 6144, DCHUNK)
            for k in range(2, 4):
                act_exp(E, psb[k - 2], k * ACHUNK, ACHUNK)
            pd2 = ppd.tile([128, DCHUNK], f32, tag="psd")
            mains(pd2, m, 6144 + DCHUNK, DCHUNK)
            finish_tree(*prev)
            dve_exp(E, pd1, 6144, DCHUNK)
            dve_exp(E, pd2, 6144 + DCHUNK, DCHUNK)
            t1b = tp.tile([128, 2048], f16, tag="t1b")
            nc.vector.tensor_tensor(
                out=t1b[:], in0=E[:, 4096:6144], in1=E[:, 6144:8192],
                op=Alu.add)
            psa = []
            for k in range(2):
                ps = ppa.tile([128, ACHUNK], f32, tag="psa")
                mains(ps, m, k * ACHUNK, ACHUNK)
                psa.append(ps)
            for k in range(2):
                act_exp(E, psa[k], k * ACHUNK, ACHUNK)
            t1a = tp.tile([128, 2048], f16, tag="t1a")
            nc.vector.tensor_tensor(
                out=t1a[:], in0=E[:, 0:2048], in1=E[:, 2048:4096],
                op=Alu.add)
            t2 = tp.tile([128, TREE_OUT], f16, tag="t2")
            for h in range(2):
                lo, hi = h * 1024, (h + 1) * 1024
                nc.vector.tensor_tensor(
                    out=t2[:, lo:hi], in0=t1a[:, lo:hi],
                    in1=t1b[:, lo:hi], op=Alu.add)
                nc.sync.dma_start(
                    out_d[:, m * TREE_OUT + lo:m * TREE_OUT + hi],
                    t2[:, lo:hi])
    nc.compile()
    return nc


def _prep_inputs(features: np.ndarray, alpha):
    feats = np.ascontiguousarray(np.asarray(features, dtype=np.float32))
    assert feats.shape == (N, D), feats.shape
    a = float(np.asarray(alpha, dtype=np.float32))

    norms = np.sqrt((feats.astype(np.float64) ** 2).sum(axis=1, keepdims=True))
    norms = np.maximum(norms, 1e-12)
    fn64 = feats / norms
    fT = np.ascontiguousarray(fn64.T.astype(np.float16))  # [128, 8192] fp16

    ident = np.eye(128, dtype=np.float16)
    negd = (np.eye(128) * DIAG_NEG).astype(np.float16)

    in_maps = []
    for ci in range(NCORES):
        ftc = np.ascontiguousarray(np.roll(fT, -ci * R, axis=1))
        in_maps.append({"ft": ftc, "ident": ident, "negd": negd})
    return in_maps, a, fn64


def _assemble(results, alpha: float, fn64: np.ndarray) -> np.float32:
    c = _shift_center(alpha)
    factor = np.exp(np.float64(c) / TAU)
    thr = 0.80 * np.exp((np.float64(alpha) - c) / TAU)

    S = np.empty(N, np.float64)
    cand = np.empty(N, bool)
    for ci in range(NCORES):
        tS = np.asarray(results[ci]["treeS"]).reshape(128, NBLK, TREE_OUT)
        S[ci * R:(ci + 1) * R] = (
            tS.astype(np.float64).sum(axis=2).T.reshape(R) * factor)
        cand[ci * R:(ci + 1) * R] = (
            (tS.astype(np.float32) >= thr).any(axis=2).T.reshape(R))

    num = np.full(N, EPS)
    den = S + 2.0 * EPS

    idx = np.flatnonzero(cand)
    if idx.size:
        sims = fn64[idx] @ fn64.T                        # [ncand, N] fp64
        e = np.exp(sims / TAU)
        e[np.arange(idx.size), idx] = 0.0
        pos = sims >= alpha
        pos[np.arange(idx.size), idx] = False
        P = (e * pos).sum(axis=1)
        Srow = e.sum(axis=1)
        num[idx] = P + EPS
        den[idx] = P + EPS + (Srow - P) + EPS
    loss = np.mean(np.log(den) - np.log(num))
    return np.float32(loss)


def kernel(features, alpha):
    from concourse.bass_utils import run_bass_kernel_spmd

    global LAST_RESULT
    in_maps, a, fn64 = _prep_inputs(features, alpha)
    if a not in _CACHE:
        _CACHE[a] = _build(a)
    nc = _CACHE[a]
    res = run_bass_kernel_spmd(
        nc, in_maps, list(range(NCORES)), trace=PROFILE
    )
    LAST_RESULT = res
    return _assemble(res.results, a, fn64)
